# revision 33
# baseline (speedup 1.0000x reference)
"""Trainium2 Bass kernel for GQA attention prefill with LoRA (+RoPE, causal).

Strategy: tensor-parallel over heads across 8 NeuronCores.
  - core c owns q-heads [4c, 4c+4) and kv-head c
  - w_q/w_k/w_v are row-sharded, w_o column-sharded; per-core partial outputs
    (full [T, DIM], fp16) are summed on the host.
  - Q/K/V LoRA folded into the weights on the host (exact); output LoRA
    computed on device (it acts on x, not on attn out) with tokens sharded
    across cores.
  - fp32r matmuls (1 cyc/row at N>=256), fp32 accumulation in PSUM.
  - RoPE handled in deinterleaved layout via host-side weight-row permutation
    (cancels in the QK inner product).
  - Attention computed in scoresT[k, q] layout so no transposes are needed
    between QK, softmax and PV; softmax skips max-subtraction (scores are
    O(10), exp is safe in fp32); column sums via ones-matmul; normalization
    via DVE reciprocal + GpSimd broadcast + DVE multiply.

v2 changes vs baseline:
  - Q stays resident in SBUF (f32r, 8MB) instead of a DRAM spill round-trip.
  - Pass B restructured: scores in 2-keyblock chunks with ONE batched exp
    per chunk (amortizes the ACT 352-cycle fixed cost), and the O-projection
    + LoRA-O matmuls are drained 2-per-chunk from a pending queue so the PE
    always has independent work while ACT grinds exp (no exp-wait stalls).
  - out / out_lo written as fp16 (halves the dominant DMA stream).
  - exp table preloaded at kernel start (hides the ~2.7us ACT table load).
"""
import os
import numpy as np
import ml_dtypes
from collections import deque

import concourse.bass as bass
import concourse.mybir as mybir
import concourse.tile as tile
from concourse import bacc
from concourse.bass_utils import run_bass_kernel_spmd

F32 = mybir.dt.float32
F32R = mybir.dt.float32r
F16 = mybir.dt.float16
BF16 = mybir.dt.bfloat16
AF = mybir.ActivationFunctionType
OP = mybir.AluOpType

B, S, DIM = 2, 2048, 4096
NH, NKV, HD = 32, 8, 128
RANK, SCALE = 16, 2.0
NCORES = 8
THETA = 500000.0

_CACHE = {}


def _build(S_=S):
    """Build the per-core SPMD program. Parameterized by sequence length for
    fast small-scale testing; everything else fixed."""
    T = B * S_                     # total tokens
    NBLK = T // 512                # 512-token blocks (pass A)
    NQB = S_ // 512                # q blocks per batch (pass B)
    NCH = DIM // 128               # 32 contraction chunks
    QD = NH * HD // NCORES         # 512 q dims per core
    NQH = QD // HD                 # 4 q heads per core
    LTOK = T // NCORES             # 512 token slice for lora-o

    nc = bacc.Bacc("TRN2", target_bir_lowering=False, debug=False)

    # ---- DRAM I/O ----
    xT_d = nc.dram_tensor("xT", [NCH, 128, T], BF16, kind="ExternalInput")
    xlo_d = nc.dram_tensor("xlo", [NCH, 128, LTOK], BF16, kind="ExternalInput")
    wq_d = nc.dram_tensor("wqT", [NCH, 128, QD], BF16, kind="ExternalInput")
    wk_d = nc.dram_tensor("wkT", [NCH, 128, HD], BF16, kind="ExternalInput")
    wv_d = nc.dram_tensor("wvT", [NCH, 128, HD], BF16, kind="ExternalInput")
    wo_d = nc.dram_tensor("woT", [NQH, 128, DIM], BF16, kind="ExternalInput")
    loa_d = nc.dram_tensor("loaT", [NCH, 128, RANK], BF16, kind="ExternalInput")
    lob_d = nc.dram_tensor("lobT", [RANK, DIM], BF16, kind="ExternalInput")
    cos_d = nc.dram_tensor("cos2", [64, T], F16, kind="ExternalInput")
    sin_d = nc.dram_tensor("sin2", [64, T], F16, kind="ExternalInput")
    onesc_d = nc.dram_tensor("onesc", [128, 1], F32R, kind="ExternalInput")
    ident_d = nc.dram_tensor("ident", [128, 128], F32R, kind="ExternalInput")
    tril_d = nc.dram_tensor("trilm", [128, 128], F32, kind="ExternalInput")

    out_d = nc.dram_tensor("out", [T, DIM], F16, kind="ExternalOutput")
    outlo_d = nc.dram_tensor("out_lo", [LTOK, DIM], F16, kind="ExternalOutput")

    from contextlib import ExitStack
    with tile.TileContext(nc) as tc, ExitStack() as es0:
        if True:
            pres = es0.enter_context(tc.tile_pool(name="res", bufs=1))
            plo = es0.enter_context(tc.tile_pool(name="losb", bufs=2))
            plob = es0.enter_context(tc.tile_pool(name="lobp", bufs=1))
            esW = ExitStack()
            pwq = esW.enter_context(tc.tile_pool(name="wqp", bufs=1))
            pwk = esW.enter_context(tc.tile_pool(name="wkp", bufs=1))
            pwv = esW.enter_context(tc.tile_pool(name="wvp", bufs=1))
            BB = NBLK // B                 # 512-blocks per batch
            kT = [pres.tile([128, S_], F16, tag=f"kT{b}", name=f"kT{b}")
                  for b in range(B)]
            vN = [pres.tile([128, S_ // 128, 128], F32R, tag=f"vN{b}",
                            name=f"vN{b}")
                  for b in range(B)]
            qB = [pres.tile([128, NQH, S_], F16, tag=f"qB{b}", name=f"qB{b}")
                  for b in range(B)]       # resident Q
            soT = pres.tile([RANK, LTOK], BF16, tag="soT")
            onesc = pres.tile([128, 1], F32R, tag="onesc")
            ident = pres.tile([128, 128], F32R, tag="ident")
            trilm = pres.tile([128, 128], F32, tag="trilm")
            warm = pres.tile([1, 1], F32, tag="warm")
            nc.sync.dma_start(onesc[:, :], onesc_d[:, :])
            nc.sync.dma_start(ident[:, :], ident_d[:, :])
            nc.sync.dma_start(trilm[:, :], tril_d[:, :])
            # preload the exp table on ACT while pass A runs
            nc.scalar.activation(warm[:, :], trilm[0:1, 0:1], AF.Exp)

            wq = pwq.tile([128, NCH, QD], BF16, tag="wq")
            wk = pwk.tile([128, NCH, HD], BF16, tag="wk")
            wv = pwv.tile([128, NCH, HD], BF16, tag="wv")

            def dma_wq(i):
                nc.sync.dma_start(
                    wq[:, i:i + 1, :],
                    wq_d[i:i + 1].rearrange("c p m -> p c m"),
                )

            def dma_wkv(i):
                c8 = NCH // 8
                nc.sync.dma_start(
                    wk[:, i * c8:(i + 1) * c8, :],
                    wk_d[i * c8:(i + 1) * c8].rearrange("c p m -> p c m"),
                )
                nc.sync.dma_start(
                    wv[:, i * c8:(i + 1) * c8, :],
                    wv_d[i * c8:(i + 1) * c8].rearrange("c p m -> p c m"),
                )

            # upfront: only what the first eighth of block 0 needs
            for i in range(4):
                dma_wq(i)
            dma_wkv(0)
            # the rest is interleaved into block 0's stream (see below)
            wdma = []
            for qt in range(1, 8):
                wdma.append([("wq", qt * 4 + j) for j in range(4)] + [("wkv", qt)])

            # ---- Pass A: projections + RoPE + V transpose ----
            esA = ExitStack()
            pxt = esA.enter_context(tc.tile_pool(name="xt", bufs=5))
            pcs = esA.enter_context(tc.tile_pool(name="cs", bufs=2))
            prt = esA.enter_context(tc.tile_pool(name="rtmp", bufs=1))
            pqc = esA.enter_context(tc.tile_pool(name="qc16", bufs=4))
            pvm = esA.enter_context(tc.tile_pool(name="vtmp", bufs=2))
            ppp = esA.enter_context(tc.tile_pool(name="pps", bufs=7, space="PSUM"))
            pvt = esA.enter_context(tc.tile_pool(name="vtps", bufs=1, space="PSUM"))
            if True:
                for blk in range(NBLK):
                    t0 = blk * 512
                    ab = blk // BB          # batch this block belongs to
                    tl = (blk % BB) * 512   # token offset within the batch
                    q_ps = [
                        ppp.tile([128, 512], F32, tag="projps", name=f"qps{qi}")
                        for qi in range(NQH)
                    ]
                    k_ps = ppp.tile([128, 512], F32, tag="projps")
                    v_ps = ppp.tile([128, 512], F32, tag="projps")
                    for qt in range(8):
                        nch8 = NCH // 8
                        xt = pxt.tile([128, nch8, 512], BF16, tag="xt")
                        nc.sync.dma_start(
                            xt[:, :, :],
                            xT_d[qt * nch8:(qt + 1) * nch8, :, t0:t0 + 512]
                            .rearrange("c p t -> p c t"),
                        )
                        if blk == 0 and qt < 7:
                            for kind, arg in wdma[qt]:
                                if kind == "wq":
                                    dma_wq(arg)
                                elif kind == "wkv":
                                    dma_wkv(arg)
                        for ch in range(nch8):
                            g = qt * nch8 + ch
                            st, sp = (g == 0), (g == NCH - 1)
                            for qi in range(NQH):
                                nc.tensor.matmul(
                                    q_ps[qi][:, :],
                                    wq[:, g, qi * 128:(qi + 1) * 128],
                                    xt[:, ch, :], start=st, stop=sp,
                                )
                            nc.tensor.matmul(
                                k_ps[:, :], wk[:, g, :], xt[:, ch, :],
                                start=st, stop=sp,
                            )
                            nc.tensor.matmul(
                                v_ps[:, :], wv[:, g, :], xt[:, ch, :],
                                start=st, stop=sp,
                            )
                    # RoPE (deinterleaved): rows 0:64 = even pairs (u),
                    # 64:128 = odd pairs (v). The PSUM accumulators are first
                    # dumped to fp16 SBUF by ACT (fast PSUM release for the
                    # next block's chains); the rotation then runs on DVE in
                    # fp16 2x mode.
                    cosb = pcs.tile([64, 512], F16, tag="cosb")
                    sinb = pcs.tile([64, 512], F16, tag="sinb")
                    nc.sync.dma_start(cosb[:, :], cos_d[:, t0:t0 + 512])
                    nc.sync.dma_start(sinb[:, :], sin_d[:, t0:t0 + 512])

                    def rope(src_ps, dst_u, dst_v, nm):
                        # u/v must land at base partition 0: DVE requires
                        # equal base partitions for two SBUF inputs. The two
                        # copies go to different engines so the PSUM bank
                        # frees after ~one copy latency.
                        uc = pqc.tile([64, 512], F16, tag="qc", name=nm + "u")
                        vc = pqc.tile([64, 512], F16, tag="qc", name=nm + "v")
                        nc.scalar.activation(uc[:, :], src_ps[0:64, :], AF.Copy)
                        nc.vector.tensor_copy(vc[:, :], src_ps[64:128, :])
                        u = uc[:, :]
                        v = vc[:, :]
                        t1 = prt.tile([64, 512], F16, tag="t1", name="t1")
                        t2 = prt.tile([64, 512], F16, tag="t2", name="t2")
                        nc.vector.tensor_tensor(t1[:, :], u, cosb[:, :], OP.mult)
                        nc.vector.tensor_tensor(t2[:, :], v, sinb[:, :], OP.mult)
                        nc.vector.tensor_tensor(dst_u, t1[:, :], t2[:, :], OP.subtract)
                        t3 = prt.tile([64, 512], F16, tag="t1", name="t3")
                        t4 = prt.tile([64, 512], F16, tag="t2", name="t4")
                        nc.vector.tensor_tensor(t3[:, :], u, sinb[:, :], OP.mult)
                        nc.vector.tensor_tensor(t4[:, :], v, cosb[:, :], OP.mult)
                        nc.vector.tensor_tensor(dst_v, t3[:, :], t4[:, :], OP.add)

                    for qi in range(NQH):
                        rope(q_ps[qi], qB[ab][0:64, qi, tl:tl + 512],
                             qB[ab][64:128, qi, tl:tl + 512], f"qc{qi}")
                    rope(k_ps, kT[ab][0:64, tl:tl + 512],
                         kT[ab][64:128, tl:tl + 512], "kc")
                    # V -> natural [tok, hd] layout via PE transpose
                    vtmp = pvm.tile([128, 512], F32R, tag="vtmp")
                    nc.scalar.activation(vtmp[:, :], v_ps[:, :], AF.Copy)
                    for st4 in range(4):
                        vt_ps = pvt.tile([128, 128], F32R, tag="vtps")
                        nc.tensor.transpose(
                            vt_ps[:, :], vtmp[:, st4 * 128:(st4 + 1) * 128],
                            ident[:, :],
                        )
                        nc.scalar.activation(
                            vN[ab][:, (blk % BB) * 4 + st4, :], vt_ps[:, :],
                            AF.Copy
                        )

            esA.close()
            esW.close()
            # ---- Pass B: attention + O-projection (+ LoRA-O) ----
            ppr = es0.enter_context(tc.tile_pool(name="prb", bufs=3))
            pat = es0.enter_context(tc.tile_pool(name="atn", bufs=3))
            prc = es0.enter_context(tc.tile_pool(name="rcp", bufs=2))
            pbc = es0.enter_context(tc.tile_pool(name="bcs", bufs=2))
            pos = es0.enter_context(tc.tile_pool(name="osb", bufs=4))
            pacc = es0.enter_context(tc.tile_pool(name="accp", bufs=2))
            pwo = es0.enter_context(tc.tile_pool(name="wop", bufs=1))
            psc = es0.enter_context(tc.tile_pool(name="scps", bufs=2, space="PSUM"))
            poh = es0.enter_context(tc.tile_pool(name="ohps", bufs=2, space="PSUM"))
            pop = es0.enter_context(tc.tile_pool(name="opps", bufs=2, space="PSUM"))
            if True:
                drains = deque()

                def drain(k):
                    for _ in range(k):
                        if drains:
                            drains.popleft()()

                # --- LoRA-O part 1: s = x_slice @ loa.T (K-chained).
                # DMAs prefetched one step ahead so the in-order PE queue
                # never reaches a matmul whose input DMA was just issued.
                # These DMAs go BEFORE the 4MB wo load: lo1 runs first.
                lo_state = {}
                n8 = NCH // 8
                # lo/wo loads ride the (idle) GpSimd queue: the Sync queue
                # serializes DMA triggers in order, and pool-gated xt
                # triggers would delay these otherwise-independent loads.
                lob = plob.tile([RANK, DIM], BF16, tag="lob", name="lob")
                nc.gpsimd.dma_start(lob[:, :], lob_d[:, :])
                lo_state["lob"] = lob
                la_all = plob.tile([128, NCH, RANK], BF16, tag="laall",
                                   name="la_all")
                nc.gpsimd.dma_start(
                    la_all[:, :, :], loa_d[:, :, :].rearrange("c p r -> p c r")
                )
                lo_xl = [None] * 9

                def dma_xl(qt):
                    xl = plo.tile([128, n8, LTOK], BF16, tag="xl", name="xl")
                    nc.gpsimd.dma_start(
                        xl[:, :, :],
                        xlo_d[qt * n8:(qt + 1) * n8, :, :]
                        .rearrange("c p t -> p c t"),
                    )
                    lo_xl[qt] = xl

                dma_xl(0)
                dma_xl(1)

                # wo arrives during block 0's attention (first needed by the
                # o-proj drains of block 1)
                wo = pwo.tile([128, NQH, DIM], BF16, tag="wo")

                def dma_wo(k):
                    nc.gpsimd.dma_start(
                        wo[:, k:k + 1, :],
                        wo_d[k:k + 1, :, :].rearrange("h p m -> p h m"),
                    )

                for k in range(NQH):
                    dma_wo(k)

                def mk_lo1(qt):
                    def emit():
                        if qt == 0:
                            lo_state["ps"] = pop.tile(
                                [RANK, LTOK], F32, tag="opps", name="lo_ps"
                            )
                        lo_ps = lo_state["ps"]
                        xl = lo_xl[qt]
                        for ch in range(n8):
                            g = qt * n8 + ch
                            nc.tensor.matmul(
                                lo_ps[:, :], la_all[:, g, :], xl[:, ch, :],
                                start=(g == 0), stop=(g == NCH - 1),
                            )
                        if qt + 2 < 8:
                            dma_xl(qt + 2)
                        if qt == 7:
                            nc.scalar.activation(soT[:, :], lo_ps[:, :], AF.Copy)
                    return emit

                for qt in range(8):
                    drains.append(mk_lo1(qt))

                # --- LoRA-O part 2 closures: outlo = s.T @ lob (per tile)
                def mk_lo2(ts4, od):
                    def emit():
                        lob = lo_state["lob"]
                        op_ps = pop.tile([128, 512], F32, tag="opps",
                                         name="lo2ps")
                        nc.tensor.matmul(
                            op_ps[:, :],
                            soT[:, ts4 * 128:(ts4 + 1) * 128],
                            lob[:, od * 512:(od + 1) * 512],
                            start=True, stop=True,
                        )
                        osb = pos.tile([128, 512], F16, tag="osb", name="lo2sb")
                        if (ts4 + od) % 2 == 0:
                            nc.scalar.activation(osb[:, :], op_ps[:, :], AF.Copy)
                        else:
                            nc.vector.tensor_copy(osb[:, :], op_ps[:, :])
                        nc.sync.dma_start(
                            outlo_d[ts4 * 128:(ts4 + 1) * 128,
                                    od * 512:(od + 1) * 512],
                            osb[:, :],
                        )
                    return emit

                # --- O-projection closures (per output tile)
                def mk_oproj(atn, g0, ts4, od):
                    def emit():
                        op_ps = pop.tile([128, 512], F32, tag="opps",
                                         name="opps")
                        for h in range(NQH):
                            nc.tensor.matmul(
                                op_ps[:, :],
                                atn[:, h, ts4 * 128:(ts4 + 1) * 128],
                                wo[:, h, od * 512:(od + 1) * 512],
                                start=(h == 0), stop=(h == NQH - 1),
                            )
                        osb = pos.tile([128, 512], F16, tag="osb", name="osb")
                        # alternate copy engines: ACT's FIFO must not hold
                        # two copies between consecutive exps, and DVE also
                        # carries the softmax-sum adds.
                        if (ts4 + od) % 2 == 0:
                            nc.scalar.activation(osb[:, :], op_ps[:, :], AF.Copy)
                        else:
                            nc.vector.tensor_copy(osb[:, :], op_ps[:, :])
                        nc.sync.dma_start(
                            out_d[g0 + ts4 * 128:g0 + (ts4 + 1) * 128,
                                  od * 512:(od + 1) * 512],
                            osb[:, :],
                        )
                    return emit

                for b in range(B):
                    for qb in range(NQB):
                        g0 = b * S_ + qb * 512   # global token of q range
                        q0 = qb * 512            # within batch
                        nkb = (qb + 1) * 4
                        nch = (nkb + 1) // 2
                        atn = pat.tile([128, NQH, 512], BF16, tag="atn")
                        for h in range(NQH):
                            oh_ps = poh.tile([128, 512], F32, tag="ohps")
                            # exp-sums accumulate on DVE instead of per-kb
                            # PE ones-matmuls; one ones-matmul per head does
                            # the final partition reduction. (GpSimd is far
                            # too slow for these adds — ~1.1us per op.)
                            acc = pacc.tile([128, 512], F32R, tag="acc")
                            eng = nc.vector

                            def emit_scores(cc):
                                kbs = list(range(2 * cc, min(2 * cc + 2, nkb)))
                                sc = psc.tile([128, 2, 512], F32, tag="scps",
                                              name="scps")
                                for j, kb in enumerate(kbs):
                                    j0 = kb - qb * 4
                                    c0 = max(j0 * 128, 0)
                                    nc.tensor.matmul(
                                        sc[:, j, c0:512],
                                        kT[b][:, kb * 128:kb * 128 + 128],
                                        qB[b][:, h, q0 + c0:q0 + 512],
                                        start=True, stop=True,
                                    )
                                    if j0 >= 0:
                                        nc.vector.tensor_tensor(
                                            sc[:, j, c0:c0 + 128],
                                            sc[:, j, c0:c0 + 128],
                                            trilm[:, :], OP.add,
                                        )
                                return sc, kbs

                            pipe = emit_scores(0)
                            for cc in range(nch):
                                sc, kbs = pipe
                                nv = len(kbs)
                                pr = ppr.tile([128, 2, 512], F32R, tag="prb")
                                nc.scalar.activation(
                                    pr[:, 0:nv, :], sc[:, 0:nv, :], AF.Exp
                                )
                                if cc + 1 < nch:
                                    pipe = emit_scores(cc + 1)
                                drain(2)
                                for j, kb in enumerate(kbs):
                                    j0 = kb - qb * 4
                                    c0 = max(j0 * 128, 0)
                                    first, last = (kb == 0), (kb == nkb - 1)
                                    nc.tensor.matmul(
                                        oh_ps[:, c0:512],
                                        vN[b][:, kb, :],
                                        pr[:, j, c0:512],
                                        start=first, stop=last,
                                    )
                                    if first:
                                        eng.tensor_copy(
                                            acc[:, :], pr[:, 0, :]
                                        )
                                    else:
                                        eng.tensor_tensor(
                                            acc[:, c0:512], acc[:, c0:512],
                                            pr[:, j, c0:512], OP.add,
                                        )
                            sm_ps = pop.tile([1, 512], F32, tag="opps",
                                             name="smps")
                            nc.tensor.matmul(
                                sm_ps[:, :], onesc[:, :], acc[:, :],
                                start=True, stop=True,
                            )
                            # normalize off the PE: 1/sums on DVE (approx),
                            # broadcast on GpSimd, multiply on DVE
                            rec = prc.tile([1, 512], F32, tag="rcp")
                            nc.vector.reciprocal_approx_fast(
                                out=rec[:, :], in_=sm_ps[:, :]
                            )
                            bcs = pbc.tile([128, 512], F32, tag="bcs")
                            nc.gpsimd.partition_broadcast(
                                bcs[:, :], rec[0:1, :], channels=128
                            )
                            nc.vector.tensor_tensor(
                                atn[:, h, :], oh_ps[:, :], bcs[:, :], OP.mult
                            )
                        for ts4 in range(4):
                            for od in range(DIM // 512):
                                drains.append(mk_oproj(atn, g0, ts4, od))
                        if b == 0 and qb == NQB - 1:
                            for ts4 in range(LTOK // 128):
                                for od in range(DIM // 512):
                                    drains.append(mk_lo2(ts4, od))
                # flush whatever is left (pure PE+DMA work, no exp to hide)
                drain(len(drains))
    nc.compile()
    return nc


def _rope_perm():
    """Deinterleave permutation within one head: new j<64 -> old 2j,
    new 64+j -> old 2j+1."""
    p = np.empty(HD, np.int64)
    p[:64] = np.arange(64) * 2
    p[64:] = np.arange(64) * 2 + 1
    return p


def _host_prep(inputs, S_=S):
    T = B * S_
    QD = NH * HD // NCORES
    KVD = NKV * HD // NCORES
    LTOK = T // NCORES
    NCH = DIM // 128

    f = lambda a: np.ascontiguousarray(a, dtype=np.float32)
    x = np.asarray(inputs["x"], np.float32)[:, :S_, :].reshape(T, DIM)
    w_q = np.asarray(inputs["w_q"], np.float32)
    w_k = np.asarray(inputs["w_k"], np.float32)
    w_v = np.asarray(inputs["w_v"], np.float32)
    w_o = np.asarray(inputs["w_o"], np.float32)

    # exact LoRA fold + softmax scale fold + RoPE deinterleave permutation
    wq_eff = w_q + SCALE * (
        np.asarray(inputs["lora_wq_b"], np.float32)
        @ np.asarray(inputs["lora_wq_a"], np.float32)
    )
    wk_eff = w_k + SCALE * (
        np.asarray(inputs["lora_wk_b"], np.float32)
        @ np.asarray(inputs["lora_wk_a"], np.float32)
    )
    wv_eff = w_v + SCALE * (
        np.asarray(inputs["lora_wv_b"], np.float32)
        @ np.asarray(inputs["lora_wv_a"], np.float32)
    )
    wq_eff = wq_eff / np.sqrt(np.float32(HD))

    perm = _rope_perm()
    qperm = (np.arange(NH)[:, None] * HD + perm[None, :]).reshape(-1)
    kperm = (np.arange(NKV)[:, None] * HD + perm[None, :]).reshape(-1)
    wq_eff = wq_eff[qperm]
    wk_eff = wk_eff[kperm]

    xT = f(x.T)                                   # [DIM, T]
    xT_b32 = xT.reshape(NCH, 128, T)
    xT_b = xT_b32.astype(ml_dtypes.bfloat16)

    # RoPE tables (from the provided freqs tensors) in token-major layout
    cosT = np.asarray(inputs["freqs_cos"], np.float32)[:S_].T  # [64, S]
    sinT = np.asarray(inputs["freqs_sin"], np.float32)[:S_].T
    cos2 = np.tile(cosT, (1, B)).astype(np.float16)   # [64, T]
    sin2 = np.tile(sinT, (1, B)).astype(np.float16)

    onesc = np.ones((128, 1), np.float32)
    ident = np.eye(128, dtype=np.float32)
    # scoresT[k, q] additive mask for diagonal 128-blocks, taken from the
    # provided mask (equals tril(-1e9, -1) for the causal reference)
    trilm = f(np.asarray(inputs["mask"], np.float32)[:128, :128].T)

    loaT = np.ascontiguousarray(
        np.asarray(inputs["lora_wo_a"], np.float32).T
    ).reshape(NCH, 128, RANK).astype(ml_dtypes.bfloat16)
    lobT = np.ascontiguousarray(
        SCALE * np.asarray(inputs["lora_wo_b"], np.float32).T
    ).astype(ml_dtypes.bfloat16)  # [16, DIM]

    in_maps = []
    for c in range(NCORES):
        wqT = f(wq_eff[c * QD:(c + 1) * QD].T).reshape(NCH, 128, QD).astype(ml_dtypes.bfloat16)
        wkT = f(wk_eff[c * KVD:(c + 1) * KVD].T).reshape(NCH, 128, KVD).astype(ml_dtypes.bfloat16)
        wvT = f(wv_eff[c * KVD:(c + 1) * KVD].T).reshape(NCH, 128, KVD).astype(ml_dtypes.bfloat16)
        woT = f(w_o[:, c * QD:(c + 1) * QD].T).reshape(
            NH // NCORES, 128, DIM
        ).astype(ml_dtypes.bfloat16)
        xlo = np.ascontiguousarray(xT_b[:, :, c * LTOK:(c + 1) * LTOK])
        in_maps.append({
            "xT": xT_b, "xlo": xlo, "wqT": wqT, "wkT": wkT, "wvT": wvT,
            "woT": woT, "loaT": loaT, "lobT": lobT,
            "cos2": cos2, "sin2": sin2,
            "onesc": onesc, "ident": ident, "trilm": trilm,
        })
    return in_maps


def run(inputs, S_=S, trace=False):
    key = S_
    if key not in _CACHE:
        _CACHE[key] = _build(S_)
    nc = _CACHE[key]
    in_maps = _host_prep(inputs, S_)
    res = run_bass_kernel_spmd(
        nc, in_maps, core_ids=list(range(NCORES)), trace=trace
    )
    T = B * S_
    LTOK = T // NCORES
    total = res.results[0]["out"].astype(np.float64)
    for c in range(1, NCORES):
        total += res.results[c]["out"]
    for c in range(NCORES):
        total[c * LTOK:(c + 1) * LTOK] += res.results[c]["out_lo"]
    out = total.astype(np.float32).reshape(B, S_, DIM)
    return out, res


def kernel(**inputs):
    out, _ = run(inputs, S)
    return out


# revision 37
# speedup vs baseline: 1.2130x; 1.2130x over previous
"""Trainium2 Bass kernel for GQA attention prefill with LoRA (+RoPE, causal).

Strategy: tensor-parallel over heads across 8 NeuronCores.
  - core c owns q-heads [4c, 4c+4) and kv-head c
  - w_q/w_k/w_v are row-sharded, w_o column-sharded; per-core partial outputs
    (full [T, DIM], fp16) are summed on the host.
  - Q/K/V LoRA folded into the weights on the host (exact); output LoRA
    computed on device (it acts on x, not on attn out) with tokens sharded
    across cores.
  - fp32r matmuls (1 cyc/row at N>=256), fp32 accumulation in PSUM.
  - RoPE handled in deinterleaved layout via host-side weight-row permutation
    (cancels in the QK inner product).
  - Attention computed in scoresT[k, q] layout so no transposes are needed
    between QK, softmax and PV; softmax skips max-subtraction (scores are
    O(10), exp is safe in fp32); column sums via ones-matmul; normalization
    via DVE reciprocal + GpSimd broadcast + DVE multiply.

v2 changes vs baseline:
  - Q stays resident in SBUF (f32r, 8MB) instead of a DRAM spill round-trip.
  - Pass B restructured: scores in 2-keyblock chunks with ONE batched exp
    per chunk (amortizes the ACT 352-cycle fixed cost), and the O-projection
    + LoRA-O matmuls are drained 2-per-chunk from a pending queue so the PE
    always has independent work while ACT grinds exp (no exp-wait stalls).
  - out / out_lo written as fp16 (halves the dominant DMA stream).
  - exp table preloaded at kernel start (hides the ~2.7us ACT table load).
"""
import os
import numpy as np
import ml_dtypes
from collections import deque

import concourse.bass as bass
import concourse.mybir as mybir
import concourse.tile as tile
from concourse import bacc
from concourse.bass_utils import run_bass_kernel_spmd

F32 = mybir.dt.float32
F32R = mybir.dt.float32r
F16 = mybir.dt.float16
BF16 = mybir.dt.bfloat16
AF = mybir.ActivationFunctionType
OP = mybir.AluOpType

B, S, DIM = 2, 2048, 4096
NH, NKV, HD = 32, 8, 128
RANK, SCALE = 16, 2.0
NCORES = 8
THETA = 500000.0

_CACHE = {}


def _build(S_=S):
    """Build the per-core SPMD program. Parameterized by sequence length for
    fast small-scale testing; everything else fixed."""
    T = B * S_                     # total tokens
    NBLK = T // 512                # 512-token blocks (pass A)
    NQB = S_ // 512                # q blocks per batch (pass B)
    NCH = DIM // 128               # 32 contraction chunks
    QD = NH * HD // NCORES         # 512 q dims per core
    NQH = QD // HD                 # 4 q heads per core
    LTOK = T // NCORES             # 512 token slice for lora-o

    nc = bacc.Bacc("TRN2", target_bir_lowering=False, debug=False)

    # ---- DRAM I/O ----
    xT_d = nc.dram_tensor("xT", [NCH, 128, T], BF16, kind="ExternalInput")
    xlo_d = nc.dram_tensor("xlo", [NCH, 128, LTOK], BF16, kind="ExternalInput")
    wq_d = nc.dram_tensor("wqT", [NCH, 128, QD], BF16, kind="ExternalInput")
    wk_d = nc.dram_tensor("wkT", [NCH, 128, HD], BF16, kind="ExternalInput")
    wv_d = nc.dram_tensor("wvT", [NCH, 128, HD], BF16, kind="ExternalInput")
    wo_d = nc.dram_tensor("woT", [NQH, 128, DIM], BF16, kind="ExternalInput")
    loa_d = nc.dram_tensor("loaT", [NCH, 128, RANK], BF16, kind="ExternalInput")
    lob_d = nc.dram_tensor("lobT", [RANK, DIM], BF16, kind="ExternalInput")
    cos_d = nc.dram_tensor("cos2", [64, T], F16, kind="ExternalInput")
    sin_d = nc.dram_tensor("sin2", [64, T], F16, kind="ExternalInput")
    onesc_d = nc.dram_tensor("onesc", [128, 1], F32R, kind="ExternalInput")
    ident_d = nc.dram_tensor("ident", [128, 128], F32R, kind="ExternalInput")
    tril_d = nc.dram_tensor("trilm", [128, 128], F32, kind="ExternalInput")

    out_d = nc.dram_tensor("out", [T, DIM], F16, kind="ExternalOutput")
    outlo_d = nc.dram_tensor("out_lo", [LTOK, DIM], F16, kind="ExternalOutput")

    from contextlib import ExitStack
    with tile.TileContext(nc) as tc, ExitStack() as es0:
        if True:
            pres = es0.enter_context(tc.tile_pool(name="res", bufs=1))
            plo = es0.enter_context(tc.tile_pool(name="losb", bufs=2))
            plob = es0.enter_context(tc.tile_pool(name="lobp", bufs=1))
            esW = ExitStack()
            pwq = esW.enter_context(tc.tile_pool(name="wqp", bufs=1))
            pwk = esW.enter_context(tc.tile_pool(name="wkp", bufs=1))
            pwv = esW.enter_context(tc.tile_pool(name="wvp", bufs=1))
            BB = NBLK // B                 # 512-blocks per batch
            kT = [pres.tile([128, S_], F16, tag=f"kT{b}", name=f"kT{b}")
                  for b in range(B)]
            vN = [pres.tile([128, S_ // 128, 128], BF16, tag=f"vN{b}",
                            name=f"vN{b}")
                  for b in range(B)]
            qB = [pres.tile([128, NQH, S_], F16, tag=f"qB{b}", name=f"qB{b}")
                  for b in range(B)]       # resident Q
            soT = pres.tile([RANK, LTOK], BF16, tag="soT")
            onesc = pres.tile([128, 1], F32R, tag="onesc")
            ident = pres.tile([128, 128], F32R, tag="ident")
            trilm = pres.tile([128, 128], F32, tag="trilm")
            warm = pres.tile([1, 1], F32, tag="warm")
            nc.sync.dma_start(onesc[:, :], onesc_d[:, :])
            nc.sync.dma_start(ident[:, :], ident_d[:, :])
            nc.sync.dma_start(trilm[:, :], tril_d[:, :])
            # preload the exp table on ACT while pass A runs
            nc.scalar.activation(warm[:, :], trilm[0:1, 0:1], AF.Exp)

            wq = pwq.tile([128, NCH, QD], BF16, tag="wq")
            wk = pwk.tile([128, NCH, HD], BF16, tag="wk")
            wv = pwv.tile([128, NCH, HD], BF16, tag="wv")

            def dma_wq(i):
                nc.sync.dma_start(
                    wq[:, i:i + 1, :],
                    wq_d[i:i + 1].rearrange("c p m -> p c m"),
                )

            def dma_wkv(i):
                c8 = NCH // 8
                nc.sync.dma_start(
                    wk[:, i * c8:(i + 1) * c8, :],
                    wk_d[i * c8:(i + 1) * c8].rearrange("c p m -> p c m"),
                )
                nc.sync.dma_start(
                    wv[:, i * c8:(i + 1) * c8, :],
                    wv_d[i * c8:(i + 1) * c8].rearrange("c p m -> p c m"),
                )

            # upfront: only what the first eighth of block 0 needs
            for i in range(4):
                dma_wq(i)
            dma_wkv(0)
            # the rest is interleaved into block 0's stream (see below)
            wdma = []
            for qt in range(1, 8):
                wdma.append([("wq", qt * 4 + j) for j in range(4)] + [("wkv", qt)])

            # ---- Pass A: projections + RoPE + V transpose ----
            esA = ExitStack()
            pxt = esA.enter_context(tc.tile_pool(name="xt", bufs=5))
            pcs = esA.enter_context(tc.tile_pool(name="cs", bufs=2))
            prt = esA.enter_context(tc.tile_pool(name="rtmp", bufs=1))
            pqc = esA.enter_context(tc.tile_pool(name="qc16", bufs=10))
            pvm = esA.enter_context(tc.tile_pool(name="vtmp", bufs=2))
            ppp = esA.enter_context(tc.tile_pool(name="pps", bufs=7, space="PSUM"))
            pvt = esA.enter_context(tc.tile_pool(name="vtps", bufs=1, space="PSUM"))
            if True:
                for blk in range(NBLK):
                    t0 = blk * 512
                    ab = blk // BB          # batch this block belongs to
                    tl = (blk % BB) * 512   # token offset within the batch
                    q_ps = [
                        ppp.tile([128, 512], F32, tag="projps", name=f"qps{qi}")
                        for qi in range(NQH)
                    ]
                    k_ps = ppp.tile([128, 512], F32, tag="projps")
                    v_ps = ppp.tile([128, 512], F32, tag="projps")
                    for qt in range(8):
                        nch8 = NCH // 8
                        xt = pxt.tile([128, nch8, 512], BF16, tag="xt")
                        nc.sync.dma_start(
                            xt[:, :, :],
                            xT_d[qt * nch8:(qt + 1) * nch8, :, t0:t0 + 512]
                            .rearrange("c p t -> p c t"),
                        )
                        if blk == 0 and qt < 7:
                            for kind, arg in wdma[qt]:
                                if kind == "wq":
                                    dma_wq(arg)
                                elif kind == "wkv":
                                    dma_wkv(arg)
                        for ch in range(nch8):
                            g = qt * nch8 + ch
                            st, sp = (g == 0), (g == NCH - 1)
                            for qi in range(NQH):
                                nc.tensor.matmul(
                                    q_ps[qi][:, :],
                                    wq[:, g, qi * 128:(qi + 1) * 128],
                                    xt[:, ch, :], start=st, stop=sp,
                                )
                            nc.tensor.matmul(
                                k_ps[:, :], wk[:, g, :], xt[:, ch, :],
                                start=st, stop=sp,
                            )
                            nc.tensor.matmul(
                                v_ps[:, :], wv[:, g, :], xt[:, ch, :],
                                start=st, stop=sp,
                            )
                    # RoPE (deinterleaved): rows 0:64 = even pairs (u),
                    # 64:128 = odd pairs (v). The PSUM accumulators are first
                    # dumped to fp16 SBUF by ACT (fast PSUM release for the
                    # next block's chains); the rotation then runs on DVE in
                    # fp16 2x mode.
                    cosb = pcs.tile([64, 512], F16, tag="cosb")
                    sinb = pcs.tile([64, 512], F16, tag="sinb")
                    nc.sync.dma_start(cosb[:, :], cos_d[:, t0:t0 + 512])
                    nc.sync.dma_start(sinb[:, :], sin_d[:, t0:t0 + 512])

                    # Phase 1: dump all five accumulators to fp16 SBUF first
                    # (uc on ACT, vc on DVE) so every PSUM bank frees after
                    # ~one copy latency — the rotations must not sit between
                    # the copies on the in-order engine queues.
                    # u/v land at base partition 0: DVE requires equal base
                    # partitions for two SBUF inputs.
                    uvs = []
                    for qi in range(NQH + 1):
                        src_ps = q_ps[qi] if qi < NQH else k_ps
                        uc = pqc.tile([64, 512], F16, tag="qc", name=f"u{qi}")
                        vc = pqc.tile([64, 512], F16, tag="qc", name=f"v{qi}")
                        nc.scalar.activation(uc[:, :], src_ps[0:64, :], AF.Copy)
                        nc.vector.tensor_copy(vc[:, :], src_ps[64:128, :])
                        uvs.append((uc, vc))

                    # Phase 2: rotations on DVE (fp16 2x mode)
                    def rope(uv, dst_u, dst_v):
                        u = uv[0][:, :]
                        v = uv[1][:, :]
                        t1 = prt.tile([64, 512], F16, tag="t1", name="t1")
                        t2 = prt.tile([64, 512], F16, tag="t2", name="t2")
                        nc.vector.tensor_tensor(t1[:, :], u, cosb[:, :], OP.mult)
                        nc.vector.tensor_tensor(t2[:, :], v, sinb[:, :], OP.mult)
                        nc.vector.tensor_tensor(dst_u, t1[:, :], t2[:, :], OP.subtract)
                        t3 = prt.tile([64, 512], F16, tag="t1", name="t3")
                        t4 = prt.tile([64, 512], F16, tag="t2", name="t4")
                        nc.vector.tensor_tensor(t3[:, :], u, sinb[:, :], OP.mult)
                        nc.vector.tensor_tensor(t4[:, :], v, cosb[:, :], OP.mult)
                        nc.vector.tensor_tensor(dst_v, t3[:, :], t4[:, :], OP.add)

                    for qi in range(NQH):
                        rope(uvs[qi], qB[ab][0:64, qi, tl:tl + 512],
                             qB[ab][64:128, qi, tl:tl + 512])
                    rope(uvs[NQH], kT[ab][0:64, tl:tl + 512],
                         kT[ab][64:128, tl:tl + 512])
                    # V -> natural [tok, hd] layout via PE transpose
                    vtmp = pvm.tile([128, 512], F32R, tag="vtmp")
                    nc.scalar.activation(vtmp[:, :], v_ps[:, :], AF.Copy)
                    for st4 in range(4):
                        vt_ps = pvt.tile([128, 128], F32R, tag="vtps")
                        nc.tensor.transpose(
                            vt_ps[:, :], vtmp[:, st4 * 128:(st4 + 1) * 128],
                            ident[:, :],
                        )
                        nc.scalar.activation(
                            vN[ab][:, (blk % BB) * 4 + st4, :], vt_ps[:, :],
                            AF.Copy
                        )

            esA.close()
            esW.close()
            # ---- Pass B: attention + O-projection (+ LoRA-O) ----
            ppr = es0.enter_context(tc.tile_pool(name="prb", bufs=3))
            pat = es0.enter_context(tc.tile_pool(name="atn", bufs=3))
            prc = es0.enter_context(tc.tile_pool(name="rcp", bufs=2))
            pbc = es0.enter_context(tc.tile_pool(name="bcs", bufs=2))
            pos = es0.enter_context(tc.tile_pool(name="osb", bufs=4))
            pacc = es0.enter_context(tc.tile_pool(name="accp", bufs=2))
            pwo = es0.enter_context(tc.tile_pool(name="wop", bufs=1))
            psc = es0.enter_context(tc.tile_pool(name="scps", bufs=2, space="PSUM"))
            poh = es0.enter_context(tc.tile_pool(name="ohps", bufs=2, space="PSUM"))
            pop = es0.enter_context(tc.tile_pool(name="opps", bufs=2, space="PSUM"))
            if True:
                drains = deque()

                def drain(k):
                    for _ in range(k):
                        if drains:
                            drains.popleft()()

                # --- LoRA-O part 1: s = x_slice @ loa.T (K-chained).
                # DMAs prefetched one step ahead so the in-order PE queue
                # never reaches a matmul whose input DMA was just issued.
                # These DMAs go BEFORE the 4MB wo load: lo1 runs first.
                lo_state = {}
                n8 = NCH // 8
                # lo/wo loads ride the (idle) GpSimd queue: the Sync queue
                # serializes DMA triggers in order, and pool-gated xt
                # triggers would delay these otherwise-independent loads.
                lo_xl = [None] * 9
                la_box = []

                def dma_xl(qt):
                    xl = plo.tile([128, n8, LTOK], BF16, tag="xl", name="xl")
                    nc.gpsimd.dma_start(
                        xl[:, :, :],
                        xlo_d[qt * n8:(qt + 1) * n8, :, :]
                        .rearrange("c p t -> p c t"),
                    )
                    lo_xl[qt] = xl

                # queue order on the (in-order) GpSimd DMA path, by first
                # use: xl0/la (lo1 at block0-chunk0), xl1, wo (o-proj drains
                # from block 1 on, ~25us in), lob (lo2, much later)
                dma_xl(0)
                la_all = plob.tile([128, NCH, RANK], BF16, tag="laall",
                                   name="la_all")
                nc.gpsimd.dma_start(
                    la_all[:, :, :], loa_d[:, :, :].rearrange("c p r -> p c r")
                )
                dma_xl(1)

                wo = pwo.tile([128, NQH, DIM], BF16, tag="wo")

                def dma_wo(k):
                    nc.gpsimd.dma_start(
                        wo[:, k:k + 1, :],
                        wo_d[k:k + 1, :, :].rearrange("h p m -> p h m"),
                    )

                for k in range(NQH):
                    dma_wo(k)
                lob = plob.tile([RANK, DIM], BF16, tag="lob", name="lob")
                nc.gpsimd.dma_start(lob[:, :], lob_d[:, :])
                lo_state["lob"] = lob

                def mk_lo1(qt):
                    def emit():
                        if qt == 0:
                            lo_state["ps"] = pop.tile(
                                [RANK, LTOK], F32, tag="opps", name="lo_ps"
                            )
                        lo_ps = lo_state["ps"]
                        xl = lo_xl[qt]
                        for ch in range(n8):
                            g = qt * n8 + ch
                            nc.tensor.matmul(
                                lo_ps[:, :], la_all[:, g, :], xl[:, ch, :],
                                start=(g == 0), stop=(g == NCH - 1),
                            )
                        if qt + 2 < 8:
                            dma_xl(qt + 2)
                        if qt == 7:
                            nc.scalar.activation(soT[:, :], lo_ps[:, :], AF.Copy)
                    return emit

                for qt in range(8):
                    drains.append(mk_lo1(qt))

                # --- LoRA-O part 2 closures: outlo = s.T @ lob (per tile)
                def mk_lo2(ts4, od):
                    def emit():
                        lob = lo_state["lob"]
                        op_ps = pop.tile([128, 512], F32, tag="opps",
                                         name="lo2ps")
                        nc.tensor.matmul(
                            op_ps[:, :],
                            soT[:, ts4 * 128:(ts4 + 1) * 128],
                            lob[:, od * 512:(od + 1) * 512],
                            start=True, stop=True,
                        )
                        osb = pos.tile([128, 512], F16, tag="osb", name="lo2sb")
                        if (ts4 + od) % 2 == 0:
                            nc.scalar.activation(osb[:, :], op_ps[:, :], AF.Copy)
                        else:
                            nc.vector.tensor_copy(osb[:, :], op_ps[:, :])
                        nc.sync.dma_start(
                            outlo_d[ts4 * 128:(ts4 + 1) * 128,
                                    od * 512:(od + 1) * 512],
                            osb[:, :],
                        )
                    return emit

                # --- O-projection closures (per output tile)
                def mk_oproj(atn, g0, ts4, od):
                    def emit():
                        op_ps = pop.tile([128, 512], F32, tag="opps",
                                         name="opps")
                        for h in range(NQH):
                            nc.tensor.matmul(
                                op_ps[:, :],
                                atn[:, h, ts4 * 128:(ts4 + 1) * 128],
                                wo[:, h, od * 512:(od + 1) * 512],
                                start=(h == 0), stop=(h == NQH - 1),
                            )
                        osb = pos.tile([128, 512], F16, tag="osb", name="osb")
                        # alternate copy engines: ACT's FIFO must not hold
                        # two copies between consecutive exps, and DVE also
                        # carries the softmax-sum adds.
                        if (ts4 + od) % 2 == 0:
                            nc.scalar.activation(osb[:, :], op_ps[:, :], AF.Copy)
                        else:
                            nc.vector.tensor_copy(osb[:, :], op_ps[:, :])
                        nc.sync.dma_start(
                            out_d[g0 + ts4 * 128:g0 + (ts4 + 1) * 128,
                                  od * 512:(od + 1) * 512],
                            osb[:, :],
                        )
                    return emit

                for b in range(B):
                    for qb in range(NQB):
                        g0 = b * S_ + qb * 512   # global token of q range
                        q0 = qb * 512            # within batch
                        nkb = (qb + 1) * 4
                        nch = (nkb + 1) // 2
                        atn = pat.tile([128, NQH, 512], BF16, tag="atn")
                        for h in range(NQH):
                            oh_ps = poh.tile([128, 512], F32, tag="ohps")
                            # exp-sums accumulate on DVE instead of per-kb
                            # PE ones-matmuls; one ones-matmul per head does
                            # the final partition reduction. (GpSimd is far
                            # too slow for these adds — ~1.1us per op.)
                            acc = pacc.tile([128, 512], F32R, tag="acc")
                            eng = nc.vector

                            def emit_scores(cc):
                                kbs = list(range(2 * cc, min(2 * cc + 2, nkb)))
                                sc = psc.tile([128, 2, 512], F32, tag="scps",
                                              name="scps")
                                for j, kb in enumerate(kbs):
                                    j0 = kb - qb * 4
                                    c0 = max(j0 * 128, 0)
                                    nc.tensor.matmul(
                                        sc[:, j, c0:512],
                                        kT[b][:, kb * 128:kb * 128 + 128],
                                        qB[b][:, h, q0 + c0:q0 + 512],
                                        start=True, stop=True,
                                    )
                                    if j0 >= 0:
                                        nc.vector.tensor_tensor(
                                            sc[:, j, c0:c0 + 128],
                                            sc[:, j, c0:c0 + 128],
                                            trilm[:, :], OP.add,
                                        )
                                return sc, kbs

                            pipe = emit_scores(0)
                            for cc in range(nch):
                                sc, kbs = pipe
                                nv = len(kbs)
                                pr = ppr.tile([128, 2, 512], BF16, tag="prb")
                                nc.scalar.activation(
                                    pr[:, 0:nv, :], sc[:, 0:nv, :], AF.Exp
                                )
                                if cc + 1 < nch:
                                    pipe = emit_scores(cc + 1)
                                drain(2)
                                for j, kb in enumerate(kbs):
                                    j0 = kb - qb * 4
                                    c0 = max(j0 * 128, 0)
                                    first, last = (kb == 0), (kb == nkb - 1)
                                    nc.tensor.matmul(
                                        oh_ps[:, c0:512],
                                        vN[b][:, kb, :],
                                        pr[:, j, c0:512],
                                        start=first, stop=last,
                                    )
                                    if first:
                                        eng.tensor_copy(
                                            acc[:, :], pr[:, 0, :]
                                        )
                                    else:
                                        eng.tensor_tensor(
                                            acc[:, c0:512], acc[:, c0:512],
                                            pr[:, j, c0:512], OP.add,
                                        )
                            sm_ps = pop.tile([1, 512], F32, tag="opps",
                                             name="smps")
                            nc.tensor.matmul(
                                sm_ps[:, :], onesc[:, :], acc[:, :],
                                start=True, stop=True,
                            )
                            # normalize off the PE: 1/sums on DVE (approx),
                            # broadcast on GpSimd, multiply on DVE
                            rec = prc.tile([1, 512], F32, tag="rcp")
                            nc.vector.reciprocal_approx_fast(
                                out=rec[:, :], in_=sm_ps[:, :]
                            )
                            bcs = pbc.tile([128, 512], F32, tag="bcs")
                            nc.gpsimd.partition_broadcast(
                                bcs[:, :], rec[0:1, :], channels=128
                            )
                            nc.vector.tensor_tensor(
                                atn[:, h, :], oh_ps[:, :], bcs[:, :], OP.mult
                            )
                        for ts4 in range(4):
                            for od in range(DIM // 512):
                                drains.append(mk_oproj(atn, g0, ts4, od))
                        if b == 0 and qb == NQB - 1:
                            for ts4 in range(LTOK // 128):
                                for od in range(DIM // 512):
                                    drains.append(mk_lo2(ts4, od))
                # flush whatever is left (pure PE+DMA work, no exp to hide)
                drain(len(drains))
    nc.compile()
    return nc


def _rope_perm():
    """Deinterleave permutation within one head: new j<64 -> old 2j,
    new 64+j -> old 2j+1."""
    p = np.empty(HD, np.int64)
    p[:64] = np.arange(64) * 2
    p[64:] = np.arange(64) * 2 + 1
    return p


def _host_prep(inputs, S_=S):
    T = B * S_
    QD = NH * HD // NCORES
    KVD = NKV * HD // NCORES
    LTOK = T // NCORES
    NCH = DIM // 128

    f = lambda a: np.ascontiguousarray(a, dtype=np.float32)
    x = np.asarray(inputs["x"], np.float32)[:, :S_, :].reshape(T, DIM)
    w_q = np.asarray(inputs["w_q"], np.float32)
    w_k = np.asarray(inputs["w_k"], np.float32)
    w_v = np.asarray(inputs["w_v"], np.float32)
    w_o = np.asarray(inputs["w_o"], np.float32)

    # exact LoRA fold + softmax scale fold + RoPE deinterleave permutation
    wq_eff = w_q + SCALE * (
        np.asarray(inputs["lora_wq_b"], np.float32)
        @ np.asarray(inputs["lora_wq_a"], np.float32)
    )
    wk_eff = w_k + SCALE * (
        np.asarray(inputs["lora_wk_b"], np.float32)
        @ np.asarray(inputs["lora_wk_a"], np.float32)
    )
    wv_eff = w_v + SCALE * (
        np.asarray(inputs["lora_wv_b"], np.float32)
        @ np.asarray(inputs["lora_wv_a"], np.float32)
    )
    wq_eff = wq_eff / np.sqrt(np.float32(HD))

    perm = _rope_perm()
    qperm = (np.arange(NH)[:, None] * HD + perm[None, :]).reshape(-1)
    kperm = (np.arange(NKV)[:, None] * HD + perm[None, :]).reshape(-1)
    wq_eff = wq_eff[qperm]
    wk_eff = wk_eff[kperm]

    xT = f(x.T)                                   # [DIM, T]
    xT_b32 = xT.reshape(NCH, 128, T)
    xT_b = xT_b32.astype(ml_dtypes.bfloat16)

    # RoPE tables (from the provided freqs tensors) in token-major layout
    cosT = np.asarray(inputs["freqs_cos"], np.float32)[:S_].T  # [64, S]
    sinT = np.asarray(inputs["freqs_sin"], np.float32)[:S_].T
    cos2 = np.tile(cosT, (1, B)).astype(np.float16)   # [64, T]
    sin2 = np.tile(sinT, (1, B)).astype(np.float16)

    onesc = np.ones((128, 1), np.float32)
    ident = np.eye(128, dtype=np.float32)
    # scoresT[k, q] additive mask for diagonal 128-blocks, taken from the
    # provided mask (equals tril(-1e9, -1) for the causal reference)
    trilm = f(np.asarray(inputs["mask"], np.float32)[:128, :128].T)

    loaT = np.ascontiguousarray(
        np.asarray(inputs["lora_wo_a"], np.float32).T
    ).reshape(NCH, 128, RANK).astype(ml_dtypes.bfloat16)
    lobT = np.ascontiguousarray(
        SCALE * np.asarray(inputs["lora_wo_b"], np.float32).T
    ).astype(ml_dtypes.bfloat16)  # [16, DIM]

    in_maps = []
    for c in range(NCORES):
        wqT = f(wq_eff[c * QD:(c + 1) * QD].T).reshape(NCH, 128, QD).astype(ml_dtypes.bfloat16)
        wkT = f(wk_eff[c * KVD:(c + 1) * KVD].T).reshape(NCH, 128, KVD).astype(ml_dtypes.bfloat16)
        wvT = f(wv_eff[c * KVD:(c + 1) * KVD].T).reshape(NCH, 128, KVD).astype(ml_dtypes.bfloat16)
        woT = f(w_o[:, c * QD:(c + 1) * QD].T).reshape(
            NH // NCORES, 128, DIM
        ).astype(ml_dtypes.bfloat16)
        xlo = np.ascontiguousarray(xT_b[:, :, c * LTOK:(c + 1) * LTOK])
        in_maps.append({
            "xT": xT_b, "xlo": xlo, "wqT": wqT, "wkT": wkT, "wvT": wvT,
            "woT": woT, "loaT": loaT, "lobT": lobT,
            "cos2": cos2, "sin2": sin2,
            "onesc": onesc, "ident": ident, "trilm": trilm,
        })
    return in_maps


def run(inputs, S_=S, trace=False):
    key = S_
    if key not in _CACHE:
        _CACHE[key] = _build(S_)
    nc = _CACHE[key]
    in_maps = _host_prep(inputs, S_)
    res = run_bass_kernel_spmd(
        nc, in_maps, core_ids=list(range(NCORES)), trace=trace
    )
    T = B * S_
    LTOK = T // NCORES
    total = res.results[0]["out"].astype(np.float64)
    for c in range(1, NCORES):
        total += res.results[c]["out"]
    for c in range(NCORES):
        total[c * LTOK:(c + 1) * LTOK] += res.results[c]["out_lo"]
    out = total.astype(np.float32).reshape(B, S_, DIM)
    return out, res


def kernel(**inputs):
    out, _ = run(inputs, S)
    return out


# revision 44
# speedup vs baseline: 1.2378x; 1.0204x over previous
"""Trainium2 Bass kernel for GQA attention prefill with LoRA (+RoPE, causal).

Strategy: tensor-parallel over heads across 8 NeuronCores.
  - core c owns q-heads [4c, 4c+4) and kv-head c
  - w_q/w_k/w_v are row-sharded, w_o column-sharded; per-core partial outputs
    (full [T, DIM], fp16) are summed on the host.
  - Q/K/V LoRA folded into the weights on the host (exact); output LoRA
    computed on device (it acts on x, not on attn out) with tokens sharded
    across cores.
  - fp32r matmuls (1 cyc/row at N>=256), fp32 accumulation in PSUM.
  - RoPE handled in deinterleaved layout via host-side weight-row permutation
    (cancels in the QK inner product).
  - Attention computed in scoresT[k, q] layout so no transposes are needed
    between QK, softmax and PV; softmax skips max-subtraction (scores are
    O(10), exp is safe in fp32); column sums via ones-matmul; normalization
    via DVE reciprocal + GpSimd broadcast + DVE multiply.

v2 changes vs baseline:
  - Q stays resident in SBUF (f32r, 8MB) instead of a DRAM spill round-trip.
  - Pass B restructured: scores in 2-keyblock chunks with ONE batched exp
    per chunk (amortizes the ACT 352-cycle fixed cost), and the O-projection
    + LoRA-O matmuls are drained 2-per-chunk from a pending queue so the PE
    always has independent work while ACT grinds exp (no exp-wait stalls).
  - out / out_lo written as fp16 (halves the dominant DMA stream).
  - exp table preloaded at kernel start (hides the ~2.7us ACT table load).
"""
import os
import numpy as np
import ml_dtypes
from collections import deque

import concourse.bass as bass
import concourse.mybir as mybir
import concourse.tile as tile
from concourse import bacc
from concourse.bass_utils import run_bass_kernel_spmd

F32 = mybir.dt.float32
F32R = mybir.dt.float32r
F16 = mybir.dt.float16
BF16 = mybir.dt.bfloat16
AF = mybir.ActivationFunctionType
OP = mybir.AluOpType

B, S, DIM = 2, 2048, 4096
NH, NKV, HD = 32, 8, 128
RANK, SCALE = 16, 2.0
NCORES = 8
THETA = 500000.0

_CACHE = {}


def _build(S_=S):
    """Build the per-core SPMD program. Parameterized by sequence length for
    fast small-scale testing; everything else fixed."""
    T = B * S_                     # total tokens
    NBLK = T // 512                # 512-token blocks (pass A)
    NQB = S_ // 512                # q blocks per batch (pass B)
    NCH = DIM // 128               # 32 contraction chunks
    QD = NH * HD // NCORES         # 512 q dims per core
    NQH = QD // HD                 # 4 q heads per core
    LTOK = T // NCORES             # 512 token slice for lora-o

    nc = bacc.Bacc("TRN2", target_bir_lowering=False, debug=False)

    # ---- DRAM I/O ----
    xT_d = nc.dram_tensor("xT", [NCH, 128, T], BF16, kind="ExternalInput")
    xlo_d = nc.dram_tensor("xlo", [NCH, 128, LTOK], BF16, kind="ExternalInput")
    wq_d = nc.dram_tensor("wqT", [NCH, 128, QD], BF16, kind="ExternalInput")
    wk_d = nc.dram_tensor("wkT", [NCH, 128, HD], BF16, kind="ExternalInput")
    wv_d = nc.dram_tensor("wvT", [NCH, 128, HD], BF16, kind="ExternalInput")
    wo_d = nc.dram_tensor("woT", [NQH, 128, DIM], BF16, kind="ExternalInput")
    loa_d = nc.dram_tensor("loaT", [NCH, 128, RANK], BF16, kind="ExternalInput")
    lob_d = nc.dram_tensor("lobT", [RANK, DIM], BF16, kind="ExternalInput")
    cos_d = nc.dram_tensor("cos2", [64, T], F16, kind="ExternalInput")
    sin_d = nc.dram_tensor("sin2", [64, T], F16, kind="ExternalInput")
    onesc_d = nc.dram_tensor("onesc", [128, 1], F32R, kind="ExternalInput")
    ident_d = nc.dram_tensor("ident", [128, 128], F32R, kind="ExternalInput")
    tril_d = nc.dram_tensor("trilm", [128, 128], F32R, kind="ExternalInput")

    out_d = nc.dram_tensor("out", [T, DIM], F16, kind="ExternalOutput")
    outlo_d = nc.dram_tensor("out_lo", [LTOK, DIM], F16, kind="ExternalOutput")

    from contextlib import ExitStack
    with tile.TileContext(nc) as tc, ExitStack() as es0:
        if True:
            pres = es0.enter_context(tc.tile_pool(name="res", bufs=1))
            plo = es0.enter_context(tc.tile_pool(name="losb", bufs=2))
            plob = es0.enter_context(tc.tile_pool(name="lobp", bufs=1))
            esW = ExitStack()
            pwq = esW.enter_context(tc.tile_pool(name="wqp", bufs=1))
            pwk = esW.enter_context(tc.tile_pool(name="wkp", bufs=1))
            pwv = esW.enter_context(tc.tile_pool(name="wvp", bufs=1))
            BB = NBLK // B                 # 512-blocks per batch
            kT = [pres.tile([128, S_], F16, tag=f"kT{b}", name=f"kT{b}")
                  for b in range(B)]
            vN = [pres.tile([128, S_ // 128, 128], BF16, tag=f"vN{b}",
                            name=f"vN{b}")
                  for b in range(B)]
            qB = [pres.tile([128, NQH, S_], F16, tag=f"qB{b}", name=f"qB{b}")
                  for b in range(B)]       # resident Q
            soT = pres.tile([RANK, LTOK], BF16, tag="soT")
            onesc = pres.tile([128, 1], F32R, tag="onesc")
            ident = pres.tile([128, 128], F32R, tag="ident")
            trilm = pres.tile([128, 128], F32R, tag="trilm")
            warm = pres.tile([1, 1], F32, tag="warm")
            nc.sync.dma_start(onesc[:, :], onesc_d[:, :])
            nc.sync.dma_start(ident[:, :], ident_d[:, :])
            nc.sync.dma_start(trilm[:, :], tril_d[:, :])
            # preload the exp table on ACT while pass A runs
            nc.scalar.activation(warm[:, :], onesc[0:1, :].bitcast(F32), AF.Exp)

            wq = pwq.tile([128, NCH, QD], BF16, tag="wq")
            wk = pwk.tile([128, NCH, HD], BF16, tag="wk")
            wv = pwv.tile([128, NCH, HD], BF16, tag="wv")

            # weight loads ride the idle GpSimd DMA queue so they never sit
            # behind pool-gated xt triggers on the serialized Sync queue
            def dma_wq(i):
                nc.gpsimd.dma_start(
                    wq[:, i:i + 1, :],
                    wq_d[i:i + 1].rearrange("c p m -> p c m"),
                )

            def dma_wkv(i):
                c8 = NCH // 8
                nc.gpsimd.dma_start(
                    wk[:, i * c8:(i + 1) * c8, :],
                    wk_d[i * c8:(i + 1) * c8].rearrange("c p m -> p c m"),
                )
                nc.gpsimd.dma_start(
                    wv[:, i * c8:(i + 1) * c8, :],
                    wv_d[i * c8:(i + 1) * c8].rearrange("c p m -> p c m"),
                )

            # upfront: only what the first eighth of block 0 needs
            for i in range(4):
                dma_wq(i)
            dma_wkv(0)
            # the rest is interleaved into block 0's stream (see below)
            wdma = []
            for qt in range(1, 8):
                wdma.append([("wq", qt * 4 + j) for j in range(4)] + [("wkv", qt)])

            # ---- Pass A: projections + RoPE + V transpose ----
            esA = ExitStack()
            pxt = esA.enter_context(tc.tile_pool(name="xt", bufs=5))
            pcs = esA.enter_context(tc.tile_pool(name="cs", bufs=2))
            prt = esA.enter_context(tc.tile_pool(name="rtmp", bufs=1))
            pqc = esA.enter_context(tc.tile_pool(name="qc16", bufs=10))
            pvm = esA.enter_context(tc.tile_pool(name="vtmp", bufs=2))
            ppp = esA.enter_context(tc.tile_pool(name="pps", bufs=7, space="PSUM"))
            pvt = esA.enter_context(tc.tile_pool(name="vtps", bufs=1, space="PSUM"))
            if True:
                # pre-warm the PE HAM clock gate during the initial DMA
                # ramp: ~3.4us of dummy matmul activity flips the clock to
                # 2.4GHz before the first real projection matmul issues.
                wmps = pvt.tile([128, 128], F32, tag="vtps", name="warmps")
                for wi in range(30):
                    nc.tensor.matmul(wmps[:, :], ident[:, :], ident[:, :],
                                     start=(wi == 0), stop=(wi == 29))
                for blk in range(NBLK):
                    t0 = blk * 512
                    ab = blk // BB          # batch this block belongs to
                    tl = (blk % BB) * 512   # token offset within the batch
                    q_ps = [
                        ppp.tile([128, 512], F32, tag="projps", name=f"qps{qi}")
                        for qi in range(NQH)
                    ]
                    k_ps = ppp.tile([128, 512], F32, tag="projps")
                    v_ps = ppp.tile([128, 512], F32, tag="projps")
                    for qt in range(8):
                        nch8 = NCH // 8
                        xt = pxt.tile([128, nch8, 512], BF16, tag="xt")
                        nc.sync.dma_start(
                            xt[:, :, :],
                            xT_d[qt * nch8:(qt + 1) * nch8, :, t0:t0 + 512]
                            .rearrange("c p t -> p c t"),
                        )
                        if blk == 0 and qt < 7:
                            for kind, arg in wdma[qt]:
                                if kind == "wq":
                                    dma_wq(arg)
                                elif kind == "wkv":
                                    dma_wkv(arg)
                        for ch in range(nch8):
                            g = qt * nch8 + ch
                            st, sp = (g == 0), (g == NCH - 1)
                            for qi in range(NQH):
                                nc.tensor.matmul(
                                    q_ps[qi][:, :],
                                    wq[:, g, qi * 128:(qi + 1) * 128],
                                    xt[:, ch, :], start=st, stop=sp,
                                )
                            nc.tensor.matmul(
                                k_ps[:, :], wk[:, g, :], xt[:, ch, :],
                                start=st, stop=sp,
                            )
                            nc.tensor.matmul(
                                v_ps[:, :], wv[:, g, :], xt[:, ch, :],
                                start=st, stop=sp,
                            )
                    # RoPE (deinterleaved): rows 0:64 = even pairs (u),
                    # 64:128 = odd pairs (v). The PSUM accumulators are first
                    # dumped to fp16 SBUF by ACT (fast PSUM release for the
                    # next block's chains); the rotation then runs on DVE in
                    # fp16 2x mode.
                    cosb = pcs.tile([64, 512], F16, tag="cosb")
                    sinb = pcs.tile([64, 512], F16, tag="sinb")
                    nc.sync.dma_start(cosb[:, :], cos_d[:, t0:t0 + 512])
                    nc.sync.dma_start(sinb[:, :], sin_d[:, t0:t0 + 512])

                    # Phase 1: dump all five accumulators to fp16 SBUF first
                    # (uc on ACT, vc on DVE) so every PSUM bank frees after
                    # ~one copy latency — the rotations must not sit between
                    # the copies on the in-order engine queues.
                    # u/v land at base partition 0: DVE requires equal base
                    # partitions for two SBUF inputs.
                    uvs = []
                    for qi in range(NQH + 1):
                        src_ps = q_ps[qi] if qi < NQH else k_ps
                        uc = pqc.tile([64, 512], F16, tag="qc", name=f"u{qi}")
                        vc = pqc.tile([64, 512], F16, tag="qc", name=f"v{qi}")
                        nc.scalar.activation(uc[:, :], src_ps[0:64, :], AF.Copy)
                        nc.vector.tensor_copy(vc[:, :], src_ps[64:128, :])
                        uvs.append((uc, vc))

                    # Phase 2: rotations on DVE (fp16 2x mode)
                    def rope(uv, dst_u, dst_v):
                        u = uv[0][:, :]
                        v = uv[1][:, :]
                        t1 = prt.tile([64, 512], F16, tag="t1", name="t1")
                        t2 = prt.tile([64, 512], F16, tag="t2", name="t2")
                        nc.vector.tensor_tensor(t1[:, :], u, cosb[:, :], OP.mult)
                        nc.vector.tensor_tensor(t2[:, :], v, sinb[:, :], OP.mult)
                        nc.vector.tensor_tensor(dst_u, t1[:, :], t2[:, :], OP.subtract)
                        t3 = prt.tile([64, 512], F16, tag="t1", name="t3")
                        t4 = prt.tile([64, 512], F16, tag="t2", name="t4")
                        nc.vector.tensor_tensor(t3[:, :], u, sinb[:, :], OP.mult)
                        nc.vector.tensor_tensor(t4[:, :], v, cosb[:, :], OP.mult)
                        nc.vector.tensor_tensor(dst_v, t3[:, :], t4[:, :], OP.add)

                    for qi in range(NQH):
                        rope(uvs[qi], qB[ab][0:64, qi, tl:tl + 512],
                             qB[ab][64:128, qi, tl:tl + 512])
                    rope(uvs[NQH], kT[ab][0:64, tl:tl + 512],
                         kT[ab][64:128, tl:tl + 512])
                    # V -> natural [tok, hd] layout via PE transpose
                    vtmp = pvm.tile([128, 512], F32R, tag="vtmp")
                    nc.scalar.activation(vtmp[:, :], v_ps[:, :], AF.Copy)
                    for st4 in range(4):
                        vt_ps = pvt.tile([128, 128], F32R, tag="vtps")
                        nc.tensor.transpose(
                            vt_ps[:, :], vtmp[:, st4 * 128:(st4 + 1) * 128],
                            ident[:, :],
                        )
                        nc.scalar.activation(
                            vN[ab][:, (blk % BB) * 4 + st4, :], vt_ps[:, :],
                            AF.Copy
                        )

            esA.close()
            esW.close()
            # ---- Pass B: attention + O-projection (+ LoRA-O) ----
            ppr = es0.enter_context(tc.tile_pool(name="prb", bufs=3))
            pat = es0.enter_context(tc.tile_pool(name="atn", bufs=3))
            prc = es0.enter_context(tc.tile_pool(name="rcp", bufs=2))
            pbc = es0.enter_context(tc.tile_pool(name="bcs", bufs=2))
            pos = es0.enter_context(tc.tile_pool(name="osb", bufs=4))
            pacc = es0.enter_context(tc.tile_pool(name="accp", bufs=2))
            pwo = es0.enter_context(tc.tile_pool(name="wop", bufs=1))
            psc = es0.enter_context(tc.tile_pool(name="scps", bufs=2, space="PSUM"))
            poh = es0.enter_context(tc.tile_pool(name="ohps", bufs=2, space="PSUM"))
            pop = es0.enter_context(tc.tile_pool(name="opps", bufs=2, space="PSUM"))
            if True:
                drains = deque()

                def drain(k):
                    for _ in range(k):
                        if drains:
                            drains.popleft()()

                # --- LoRA-O part 1: s = x_slice @ loa.T (K-chained).
                # DMAs prefetched one step ahead so the in-order PE queue
                # never reaches a matmul whose input DMA was just issued.
                # These DMAs go BEFORE the 4MB wo load: lo1 runs first.
                lo_state = {}
                n8 = NCH // 8
                # lo/wo loads ride the (idle) GpSimd queue: the Sync queue
                # serializes DMA triggers in order, and pool-gated xt
                # triggers would delay these otherwise-independent loads.
                lo_xl = [None] * 9
                la_box = []

                def dma_xl(qt):
                    xl = plo.tile([128, n8, LTOK], BF16, tag="xl", name="xl")
                    nc.gpsimd.dma_start(
                        xl[:, :, :],
                        xlo_d[qt * n8:(qt + 1) * n8, :, :]
                        .rearrange("c p t -> p c t"),
                    )
                    lo_xl[qt] = xl

                # queue order on the (in-order) GpSimd DMA path, by first
                # use: xl0/la (lo1 at block0-chunk0), xl1, wo (o-proj drains
                # from block 1 on, ~25us in), lob (lo2, much later)
                dma_xl(0)
                la_all = plob.tile([128, NCH, RANK], BF16, tag="laall",
                                   name="la_all")
                nc.gpsimd.dma_start(
                    la_all[:, :, :], loa_d[:, :, :].rearrange("c p r -> p c r")
                )
                dma_xl(1)

                wo = pwo.tile([128, NQH, DIM], BF16, tag="wo")

                def dma_wo(k):
                    nc.gpsimd.dma_start(
                        wo[:, k:k + 1, :],
                        wo_d[k:k + 1, :, :].rearrange("h p m -> p h m"),
                    )

                for k in range(NQH):
                    dma_wo(k)
                lob = plob.tile([RANK, DIM], BF16, tag="lob", name="lob")
                nc.gpsimd.dma_start(lob[:, :], lob_d[:, :])
                lo_state["lob"] = lob

                def mk_lo1(qt):
                    def emit():
                        if qt == 0:
                            lo_state["ps"] = pop.tile(
                                [RANK, LTOK], F32, tag="opps", name="lo_ps"
                            )
                        lo_ps = lo_state["ps"]
                        xl = lo_xl[qt]
                        for ch in range(n8):
                            g = qt * n8 + ch
                            nc.tensor.matmul(
                                lo_ps[:, :], la_all[:, g, :], xl[:, ch, :],
                                start=(g == 0), stop=(g == NCH - 1),
                            )
                        if qt + 2 < 8:
                            dma_xl(qt + 2)
                        if qt == 7:
                            nc.scalar.activation(soT[:, :], lo_ps[:, :], AF.Copy)
                    return emit

                for qt in range(8):
                    drains.append(mk_lo1(qt))

                # --- LoRA-O part 2 closures: outlo = s.T @ lob (per tile)
                def mk_lo2(ts4, od):
                    def emit():
                        lob = lo_state["lob"]
                        op_ps = pop.tile([128, 512], F32, tag="opps",
                                         name="lo2ps")
                        nc.tensor.matmul(
                            op_ps[:, :],
                            soT[:, ts4 * 128:(ts4 + 1) * 128],
                            lob[:, od * 512:(od + 1) * 512],
                            start=True, stop=True,
                        )
                        osb = pos.tile([128, 512], F16, tag="osb", name="lo2sb")
                        if (ts4 + od) % 2 == 0:
                            nc.scalar.activation(osb[:, :], op_ps[:, :], AF.Copy)
                        else:
                            nc.vector.tensor_copy(osb[:, :], op_ps[:, :])
                        nc.sync.dma_start(
                            outlo_d[ts4 * 128:(ts4 + 1) * 128,
                                    od * 512:(od + 1) * 512],
                            osb[:, :],
                        )
                    return emit

                # --- O-projection closures (per output tile)
                def mk_oproj(atn, g0, ts4, od):
                    def emit():
                        op_ps = pop.tile([128, 512], F32, tag="opps",
                                         name="opps")
                        for h in range(NQH):
                            nc.tensor.matmul(
                                op_ps[:, :],
                                atn[:, h, ts4 * 128:(ts4 + 1) * 128],
                                wo[:, h, od * 512:(od + 1) * 512],
                                start=(h == 0), stop=(h == NQH - 1),
                            )
                        osb = pos.tile([128, 512], F16, tag="osb", name="osb")
                        # alternate copy engines: ACT's FIFO must not hold
                        # two copies between consecutive exps, and DVE also
                        # carries the softmax-sum adds.
                        if (ts4 + od) % 2 == 0:
                            nc.scalar.activation(osb[:, :], op_ps[:, :], AF.Copy)
                        else:
                            nc.vector.tensor_copy(osb[:, :], op_ps[:, :])
                        nc.sync.dma_start(
                            out_d[g0 + ts4 * 128:g0 + (ts4 + 1) * 128,
                                  od * 512:(od + 1) * 512],
                            osb[:, :],
                        )
                    return emit

                for b in range(B):
                    for qb in range(NQB):
                        g0 = b * S_ + qb * 512   # global token of q range
                        q0 = qb * 512            # within batch
                        nkb = (qb + 1) * 4
                        nch = (nkb + 1) // 2
                        atn = pat.tile([128, NQH, 512], BF16, tag="atn")
                        for h in range(NQH):
                            oh_ps = poh.tile([128, 512], F32, tag="ohps")
                            # exp-sums accumulate on DVE instead of per-kb
                            # PE ones-matmuls; one ones-matmul per head does
                            # the final partition reduction. (GpSimd is far
                            # too slow for these adds — ~1.1us per op.)
                            acc = pacc.tile([128, 512], F32R, tag="acc")
                            eng = nc.vector

                            def emit_scores(cc):
                                kbs = list(range(2 * cc, min(2 * cc + 2, nkb)))
                                sc = psc.tile([128, 2, 512], F32, tag="scps",
                                              name="scps")
                                for j, kb in enumerate(kbs):
                                    j0 = kb - qb * 4
                                    c0 = max(j0 * 128, 0)
                                    diag = j0 >= 0
                                    nc.tensor.matmul(
                                        sc[:, j, c0:512],
                                        kT[b][:, kb * 128:kb * 128 + 128],
                                        qB[b][:, h, q0 + c0:q0 + 512],
                                        start=True, stop=not diag,
                                    )
                                    if diag:
                                        # causal mask folded in on the PE:
                                        # sc += ident.T @ trilm (= trilm)
                                        nc.tensor.matmul(
                                            sc[:, j, c0:c0 + 128],
                                            ident[:, :], trilm[:, :],
                                            start=False, stop=True,
                                        )
                                return sc, kbs

                            pipe = emit_scores(0)
                            for cc in range(nch):
                                sc, kbs = pipe
                                nv = len(kbs)
                                pr = ppr.tile([128, 2, 512], BF16, tag="prb")
                                nc.scalar.activation(
                                    pr[:, 0:nv, :], sc[:, 0:nv, :], AF.Exp
                                )
                                if cc + 1 < nch:
                                    pipe = emit_scores(cc + 1)
                                drain(2)
                                for j, kb in enumerate(kbs):
                                    j0 = kb - qb * 4
                                    c0 = max(j0 * 128, 0)
                                    first, last = (kb == 0), (kb == nkb - 1)
                                    nc.tensor.matmul(
                                        oh_ps[:, c0:512],
                                        vN[b][:, kb, :],
                                        pr[:, j, c0:512],
                                        start=first, stop=last,
                                    )
                                    if first:
                                        eng.tensor_copy(
                                            acc[:, :], pr[:, 0, :]
                                        )
                                    else:
                                        eng.tensor_tensor(
                                            acc[:, c0:512], acc[:, c0:512],
                                            pr[:, j, c0:512], OP.add,
                                        )
                            sm_ps = pop.tile([1, 512], F32, tag="opps",
                                             name="smps")
                            nc.tensor.matmul(
                                sm_ps[:, :], onesc[:, :], acc[:, :],
                                start=True, stop=True,
                            )
                            # normalize off the PE: 1/sums on DVE (approx),
                            # broadcast on GpSimd, multiply on DVE
                            rec = prc.tile([1, 512], F32, tag="rcp")
                            nc.vector.reciprocal_approx_fast(
                                out=rec[:, :], in_=sm_ps[:, :]
                            )
                            bcs = pbc.tile([128, 512], F32, tag="bcs")
                            nc.gpsimd.partition_broadcast(
                                bcs[:, :], rec[0:1, :], channels=128
                            )
                            nc.vector.tensor_tensor(
                                atn[:, h, :], oh_ps[:, :], bcs[:, :], OP.mult
                            )
                        for ts4 in range(4):
                            for od in range(DIM // 512):
                                drains.append(mk_oproj(atn, g0, ts4, od))
                        if b == 0 and qb == NQB - 1:
                            for ts4 in range(LTOK // 128):
                                for od in range(DIM // 512):
                                    drains.append(mk_lo2(ts4, od))
                # flush whatever is left (pure PE+DMA work, no exp to hide)
                drain(len(drains))
    nc.compile()
    return nc


def _rope_perm():
    """Deinterleave permutation within one head: new j<64 -> old 2j,
    new 64+j -> old 2j+1."""
    p = np.empty(HD, np.int64)
    p[:64] = np.arange(64) * 2
    p[64:] = np.arange(64) * 2 + 1
    return p


def _host_prep(inputs, S_=S):
    T = B * S_
    QD = NH * HD // NCORES
    KVD = NKV * HD // NCORES
    LTOK = T // NCORES
    NCH = DIM // 128

    f = lambda a: np.ascontiguousarray(a, dtype=np.float32)
    x = np.asarray(inputs["x"], np.float32)[:, :S_, :].reshape(T, DIM)
    w_q = np.asarray(inputs["w_q"], np.float32)
    w_k = np.asarray(inputs["w_k"], np.float32)
    w_v = np.asarray(inputs["w_v"], np.float32)
    w_o = np.asarray(inputs["w_o"], np.float32)

    # exact LoRA fold + softmax scale fold + RoPE deinterleave permutation
    wq_eff = w_q + SCALE * (
        np.asarray(inputs["lora_wq_b"], np.float32)
        @ np.asarray(inputs["lora_wq_a"], np.float32)
    )
    wk_eff = w_k + SCALE * (
        np.asarray(inputs["lora_wk_b"], np.float32)
        @ np.asarray(inputs["lora_wk_a"], np.float32)
    )
    wv_eff = w_v + SCALE * (
        np.asarray(inputs["lora_wv_b"], np.float32)
        @ np.asarray(inputs["lora_wv_a"], np.float32)
    )
    wq_eff = wq_eff / np.sqrt(np.float32(HD))

    perm = _rope_perm()
    qperm = (np.arange(NH)[:, None] * HD + perm[None, :]).reshape(-1)
    kperm = (np.arange(NKV)[:, None] * HD + perm[None, :]).reshape(-1)
    wq_eff = wq_eff[qperm]
    wk_eff = wk_eff[kperm]

    xT = f(x.T)                                   # [DIM, T]
    xT_b32 = xT.reshape(NCH, 128, T)
    xT_b = xT_b32.astype(ml_dtypes.bfloat16)

    # RoPE tables (from the provided freqs tensors) in token-major layout
    cosT = np.asarray(inputs["freqs_cos"], np.float32)[:S_].T  # [64, S]
    sinT = np.asarray(inputs["freqs_sin"], np.float32)[:S_].T
    cos2 = np.tile(cosT, (1, B)).astype(np.float16)   # [64, T]
    sin2 = np.tile(sinT, (1, B)).astype(np.float16)

    onesc = np.ones((128, 1), np.float32)
    ident = np.eye(128, dtype=np.float32)
    # scoresT[k, q] additive mask for diagonal 128-blocks, taken from the
    # provided mask (equals tril(-1e9, -1) for the causal reference)
    trilm = f(np.asarray(inputs["mask"], np.float32)[:128, :128].T)

    loaT = np.ascontiguousarray(
        np.asarray(inputs["lora_wo_a"], np.float32).T
    ).reshape(NCH, 128, RANK).astype(ml_dtypes.bfloat16)
    lobT = np.ascontiguousarray(
        SCALE * np.asarray(inputs["lora_wo_b"], np.float32).T
    ).astype(ml_dtypes.bfloat16)  # [16, DIM]

    in_maps = []
    for c in range(NCORES):
        wqT = f(wq_eff[c * QD:(c + 1) * QD].T).reshape(NCH, 128, QD).astype(ml_dtypes.bfloat16)
        wkT = f(wk_eff[c * KVD:(c + 1) * KVD].T).reshape(NCH, 128, KVD).astype(ml_dtypes.bfloat16)
        wvT = f(wv_eff[c * KVD:(c + 1) * KVD].T).reshape(NCH, 128, KVD).astype(ml_dtypes.bfloat16)
        woT = f(w_o[:, c * QD:(c + 1) * QD].T).reshape(
            NH // NCORES, 128, DIM
        ).astype(ml_dtypes.bfloat16)
        xlo = np.ascontiguousarray(xT_b[:, :, c * LTOK:(c + 1) * LTOK])
        in_maps.append({
            "xT": xT_b, "xlo": xlo, "wqT": wqT, "wkT": wkT, "wvT": wvT,
            "woT": woT, "loaT": loaT, "lobT": lobT,
            "cos2": cos2, "sin2": sin2,
            "onesc": onesc, "ident": ident, "trilm": trilm,
        })
    return in_maps


def run(inputs, S_=S, trace=False):
    key = S_
    if key not in _CACHE:
        _CACHE[key] = _build(S_)
    nc = _CACHE[key]
    in_maps = _host_prep(inputs, S_)
    res = run_bass_kernel_spmd(
        nc, in_maps, core_ids=list(range(NCORES)), trace=trace
    )
    T = B * S_
    LTOK = T // NCORES
    total = res.results[0]["out"].astype(np.float64)
    for c in range(1, NCORES):
        total += res.results[c]["out"]
    for c in range(NCORES):
        total[c * LTOK:(c + 1) * LTOK] += res.results[c]["out_lo"]
    out = total.astype(np.float32).reshape(B, S_, DIM)
    return out, res


def kernel(**inputs):
    out, _ = run(inputs, S)
    return out
